# revision 1
# baseline (speedup 1.0000x reference)
"""Trainium2 Bass kernel for a 2-layer LSTM (B=256, T=512, D=64, H=512) + FC on last step.

Sharding: data-parallel over batch — 32 samples per NeuronCore on 8 cores.
Per-core design (everything SBUF-resident, no HBM traffic inside the loop):
  - gates layout: [batch=32 partitions, 4H=2048 free], computed on PE as
    gates = state.T @ W with the (small) state as the stationary operand and
    the (large) weights as the moving operand in N=512 chunks (full stream rate).
  - recurrent state h kept transposed ([H on partitions, batch on free]) so it
    can feed the next matmul as lhsT; rebuilt each step via 4 PE transposes.
  - biases folded in: layer0 via an appended ones-row on x.T (K=65 chunk),
    layer1 via a K=1 ones-row matmul against a bias row.
  - matmuls/h in bf16, cell state c and gate nonlinearities in f32.
"""

import numpy as np
import ml_dtypes

import concourse.bass as bass
import concourse.mybir as mybir
import concourse.tile as tile
from concourse.bass_utils import run_bass_kernel_spmd
from concourse.masks import make_identity

BF16 = mybir.dt.bfloat16
F32 = mybir.dt.float32

B, T, D, H, O = 256, 512, 64, 512, 1
G = 4 * H  # 2048
NCORES = 8
BL = B // NCORES  # 32
NK_H = H // 128  # 4 K-chunks for an H-sized contraction
NN = G // 512  # 4 N-chunks of 512 gate columns
SIG = mybir.ActivationFunctionType.Sigmoid
TANH = mybir.ActivationFunctionType.Tanh


def _split_excess_waits(nc, max_waits: int = 1) -> int:
    """This container's walrus rejects >1 sync wait per instruction; move
    excess waits onto preceding same-engine NOPs (same-engine earlier wait
    is ordering-equivalent)."""
    n_split = 0
    for f in nc.m.functions:
        for bb in f.blocks:
            new_insts = []
            for inst in bb.instructions:
                si = inst.sync_info
                if si is not None and si.on_wait and len(si.on_wait) > max_waits:
                    waits = list(si.on_wait)
                    while len(waits) > max_waits:
                        chunk, waits = waits[:max_waits], waits[max_waits:]
                        nop = mybir.InstNoOp(
                            name=f"{inst.name}-wsplit-{n_split}", ins=[], outs=[]
                        )
                        nop.engine = inst.engine
                        nop.sync_info = mybir.SyncInfo(on_wait=chunk, on_update=[])
                        new_insts.append(nop)
                        n_split += 1
                    si.on_wait = waits
                new_insts.append(inst)
            bb.instructions[:] = new_insts
    return n_split


BLOCK_S = 32  # steps per layer-1 input-projection batch (V2 path)


def _default_version() -> int:
    import os

    return int(os.environ.get("LSTM_KERNEL_VERSION", "2"))


def build_lstm_nc(t_steps: int = T, version: int | None = None):
    if version is None:
        version = _default_version()
    nc = bass.Bass("TRN2")

    xt_d = nc.dram_tensor("xt", [D + 1, t_steps, BL], BF16, kind="ExternalInput")
    w0a_d = nc.dram_tensor("w0a", [D + 1, G], BF16, kind="ExternalInput")
    w0b_d = nc.dram_tensor("w0b", [128, NK_H, G], BF16, kind="ExternalInput")
    w1_d = nc.dram_tensor("w1", [128, 2 * NK_H, G], BF16, kind="ExternalInput")
    w1bias_d = nc.dram_tensor("w1bias", [1, G], BF16, kind="ExternalInput")
    fcw_d = nc.dram_tensor("fcw", [128, NK_H], BF16, kind="ExternalInput")
    ident4_d = nc.dram_tensor("ident4", [128, BL], BF16, kind="ExternalInput")
    fcb_d = nc.dram_tensor("fcb", [1, 1], F32, kind="ExternalInput")
    y_d = nc.dram_tensor("y", [BL, O], F32, kind="ExternalOutput")

    with tile.TileContext(nc) as tc:
        with (
            tc.tile_pool(name="singles", bufs=1) as singles,
            tc.tile_pool(name="state", bufs=1) as state,
            tc.tile_pool(name="work", bufs=3) as work,
            tc.tile_pool(name="psum", bufs=8, space="PSUM") as psum,
        ):
            # --- resident constants ---
            xt_s = singles.tile([D + 1, t_steps, BL], BF16)
            nc.sync.dma_start(out=xt_s, in_=xt_d[:, :, :])
            w0a_s = singles.tile([D + 1, G], BF16)
            nc.sync.dma_start(out=w0a_s, in_=w0a_d[:, :])
            w0b_s = singles.tile([128, NK_H, G], BF16)
            nc.sync.dma_start(out=w0b_s, in_=w0b_d[:, :, :])
            w1_s = singles.tile([128, 2 * NK_H, G], BF16)
            nc.sync.dma_start(out=w1_s, in_=w1_d[:, :, :])
            w1b_s = singles.tile([1, G], BF16)
            nc.sync.dma_start(out=w1b_s, in_=w1bias_d[:, :])
            fcw_s = singles.tile([128, NK_H], BF16)
            nc.sync.dma_start(out=fcw_s, in_=fcw_d[:, :])
            fcb_s = singles.tile([BL, 1], F32)
            nc.sync.dma_start(out=fcb_s, in_=fcb_d[:, :].to_broadcast((BL, 1)))
            ident = singles.tile([BL, BL], BF16)
            make_identity(nc, ident)
            ones_r = singles.tile([1, BL], BF16)
            nc.vector.memset(ones_r, 1.0)
            ones_r128 = singles.tile([1, 128], BF16)
            nc.vector.memset(ones_r128, 1.0)
            ident4_s = singles.tile([128, BL], BF16)
            nc.sync.dma_start(out=ident4_s, in_=ident4_d[:, :])

            # --- recurrent state ---
            h0T = state.tile([128, NK_H, BL], BF16)
            h1T = state.tile([128, NK_H, BL], BF16)
            c0 = state.tile([BL, H], F32)
            c1 = state.tile([BL, H], F32)
            for st in (h0T, h1T, c0, c1):
                nc.vector.memset(st, 0.0)

            def lstm_step(t, hT, cell, w_ih_first, w_s, kslices):
                """One LSTM cell update in gates-[BL, G]-layout.

                w_ih_first: (lhsT, rhs_tile) for the leading K-chunk
                  (x+ones row for layer0 / ones-row bias for layer1 /
                  identity+xp1 inject for layer1-V2).
                kslices: list of (lhsT_tile, k_index_in_w_s) for the
                  remaining accumulation chunks.
                """
                gch = []
                for n in range(NN):
                    ns = slice(n * 512, (n + 1) * 512)
                    gn = psum.tile([BL, 512], F32, tag="ps")
                    if callable(w_ih_first):
                        lhsT0, rhs0, tpos = w_ih_first(n)
                    else:
                        lhsT0, rhs0, tpos = (
                            w_ih_first[0], w_ih_first[1][:, ns], None)
                    nc.tensor.matmul(
                        gn, lhsT0, rhs0, start=True, stop=False,
                        tile_position=tpos,
                    )
                    for j, (lhsT_k, wk) in enumerate(kslices):
                        nc.tensor.matmul(
                            gn,
                            lhsT_k,
                            w_s[:, wk, ns],
                            start=False,
                            stop=(j == len(kslices) - 1),
                        )
                    gch.append(gn)

                sig_i = work.tile([BL, 512], F32, tag="sig_i")
                sig_f = work.tile([BL, 512], F32, tag="sig_f")
                tanh_g = work.tile([BL, 512], F32, tag="tanh_g")
                sig_o = work.tile([BL, 512], F32, tag="sig_o")
                nc.scalar.activation(sig_i, gch[0], SIG)
                nc.scalar.activation(sig_f, gch[1], SIG)
                nc.scalar.activation(tanh_g, gch[2], TANH)
                nc.scalar.activation(sig_o, gch[3], SIG)

                ig = work.tile([BL, 512], F32, tag="ig")
                nc.vector.tensor_mul(ig, sig_i, tanh_g)
                nc.vector.tensor_mul(cell, cell, sig_f)
                nc.vector.tensor_add(cell, cell, ig)
                tanh_c = work.tile([BL, 512], F32, tag="tanh_c")
                nc.scalar.activation(tanh_c, cell, TANH)
                h_new = work.tile([BL, H], BF16, tag="h_new")
                nc.vector.tensor_mul(h_new, sig_o, tanh_c)

                # transpose h_new [32, 512] -> hT [128, 4, 32]
                tp = psum.tile([128, NK_H, BL], BF16, tag="ps")
                for k in range(NK_H):
                    nc.tensor.transpose(
                        tp[:, k, :], h_new[:, k * 128 : (k + 1) * 128], ident
                    )
                nc.vector.tensor_copy(hT, tp)

            if version == 1:
                for t in range(t_steps):
                    lstm_step(
                        t,
                        h0T,
                        c0,
                        (xt_s[:, t, :], w0a_s),
                        w0b_s,
                        [(h0T[:, k, :], k) for k in range(NK_H)],
                    )
                    lstm_step(
                        t,
                        h1T,
                        c1,
                        (ones_r, w1b_s),
                        w1_s,
                        [(h0T[:, k, :], k) for k in range(NK_H)]
                        + [(h1T[:, k, :], NK_H + k) for k in range(NK_H)],
                    )
            else:
                # V2: per block of BLOCK_S steps — run layer0 alone collecting
                # transposed h0 into a block buffer, bulk-GEMM layer1's input
                # projection at full M=128 PE utilization, then run layer1's
                # recurrence with the projection injected via a K=32 identity
                # matmul.
                SB = BLOCK_S
                assert t_steps % SB == 0 and SB % 4 == 0
                h0blk = state.tile([128, NK_H, SB, BL], BF16)
                xp1blk = state.tile([128, SB // 4, NN, 512], BF16)
                for b in range(t_steps // SB):
                    for s in range(SB):
                        t = b * SB + s
                        prev = (
                            h0T if s == 0
                            else h0blk[:, :, s - 1, :]
                        )
                        lstm_step(
                            t,
                            h0blk[:, :, s, :],
                            c0,
                            (xt_s[:, t, :], w0a_s),
                            w0b_s,
                            [(prev[:, k, :], k) for k in range(NK_H)],
                        )
                    nc.vector.tensor_copy(h0T, h0blk[:, :, SB - 1, :])
                    for m in range(SB // 4):
                        for n in range(NN):
                            ns = slice(n * 512, (n + 1) * 512)
                            xp = psum.tile([128, 512], F32, tag="ps")
                            nc.tensor.matmul(
                                xp, ones_r128, w1b_s[:, ns],
                                start=True, stop=False,
                            )
                            for k in range(NK_H):
                                nc.tensor.matmul(
                                    xp,
                                    h0blk[:, k, 4 * m : 4 * m + 4, :].rearrange(
                                        "p a b -> p (a b)"
                                    ),
                                    w1_s[:, k, ns],
                                    start=False,
                                    stop=(k == NK_H - 1),
                                )
                            nc.vector.tensor_copy(xp1blk[:, m, n, :], xp)
                    for s in range(SB):
                        t = b * SB + s
                        lstm_step(
                            t,
                            h1T,
                            c1,
                            lambda n, s=s: (
                                ident4_s[(s % 4) * BL : (s % 4 + 1) * BL, :],
                                xp1blk[
                                    (s % 4) * BL : (s % 4 + 1) * BL, s // 4, n, :
                                ],
                                ((s % 4) * BL, 0) if s % 4 == 3 else None,
                            ),
                            w1_s,
                            [(h1T[:, k, :], NK_H + k) for k in range(NK_H)],
                        )

            # --- fc on last h1 ---
            fcp = psum.tile([BL, O], F32, tag="ps")
            for k in range(NK_H):
                nc.tensor.matmul(
                    fcp,
                    h1T[:, k, :],
                    fcw_s[:, k : k + 1],
                    start=(k == 0),
                    stop=(k == NK_H - 1),
                )
            y_s = work.tile([BL, O], F32, tag="y")
            nc.vector.tensor_add(y_s, fcp, fcb_s)
            nc.sync.dma_start(out=y_d[:, :], in_=y_s)

    _split_excess_waits(nc)
    return nc


def prep_inputs(x, w_ih_0, w_hh_0, b_ih_0, b_hh_0, w_ih_1, w_hh_1, b_ih_1, b_hh_1,
                fc_w, fc_b, t_steps: int = T):
    """Host-side layout prep + sharding. Returns per-core in_maps."""
    bf = ml_dtypes.bfloat16
    w0a = np.concatenate(
        [w_ih_0.T, (b_ih_0 + b_hh_0)[None, :]], axis=0
    ).astype(bf)  # [65, G]
    w0b = np.ascontiguousarray(
        w_hh_0.T.reshape(NK_H, 128, G).transpose(1, 0, 2)
    ).astype(bf)  # [128, 4, G]
    w1 = np.ascontiguousarray(
        np.concatenate([w_ih_1.T, w_hh_1.T], axis=0)
        .reshape(2 * NK_H, 128, G)
        .transpose(1, 0, 2)
    ).astype(bf)  # [128, 8, G]
    w1bias = (b_ih_1 + b_hh_1)[None, :].astype(bf)  # [1, G]
    fcw = np.ascontiguousarray(fc_w.reshape(NK_H, 128).T).astype(bf)  # [128, 4]
    fcb = fc_b.reshape(1, 1).astype(np.float32)
    ident4 = np.concatenate([np.eye(BL, dtype=np.float32)] * 4, axis=0).astype(bf)

    in_maps = []
    for c in range(NCORES):
        xc = x[c * BL : (c + 1) * BL, :t_steps, :]  # [32, T, 64]
        xt = np.transpose(xc, (2, 1, 0))  # [64, T, 32]
        xt = np.concatenate([xt, np.ones((1, t_steps, BL), np.float32)], axis=0)
        in_maps.append(
            {
                "xt": np.ascontiguousarray(xt).astype(bf),
                "w0a": w0a,
                "w0b": w0b,
                "w1": w1,
                "w1bias": w1bias,
                "fcw": fcw,
                "fcb": fcb,
                "ident4": ident4,
            }
        )
    return in_maps


_NC_CACHE = {}


def kernel(x, w_ih_0, w_hh_0, b_ih_0, b_hh_0, w_ih_1, w_hh_1, b_ih_1, b_hh_1,
           fc_w, fc_b):
    x = np.asarray(x, np.float32)
    args = [np.asarray(a, np.float32) for a in (
        w_ih_0, w_hh_0, b_ih_0, b_hh_0, w_ih_1, w_hh_1, b_ih_1, b_hh_1, fc_w, fc_b)]
    if T not in _NC_CACHE:
        _NC_CACHE[T] = build_lstm_nc(T)
    nc = _NC_CACHE[T]
    in_maps = prep_inputs(x, *args, t_steps=T)
    res = run_bass_kernel_spmd(nc, in_maps, core_ids=list(range(NCORES)))
    return np.concatenate([res.results[c]["y"] for c in range(NCORES)], axis=0)



# revision 2
# speedup vs baseline: 1.2274x; 1.2274x over previous
"""Trainium2 Bass kernel for a 2-layer LSTM (B=256, T=512, D=64, H=512) + FC on last step.

Sharding: data-parallel over batch - 32 samples per NeuronCore on 8 cores.

Per-core design (all weights/activations SBUF-resident):
  - gates PSUM layout [128, 512]: partition = 32*q + b (q = H-quarter, b = batch),
    free = (gate in order i,f,o,g) x (128 H-cols of quarter q). Computed with
    4-way column-tiled matmuls (tile_position=(0,32q)): four concurrent 32-col
    PE tiles, each streaming its own 512 weight columns, so a K=128 chunk
    costs ~512 PE cycles of wall for all 2048 gate columns.
  - recurrent state h kept as hT32 [128, (jj,b)]: partition p = 32q+cc holds
    H index 128q + 32jj + cc at free offset 32jj+b. Produced each step by a
    single DVE 32x32-block transpose (nc.vector.transpose) of
    h_new [128=(q,b), 128=(jj,cc)] - no PE-mode switches, no PE transposes.
  - weight rows are host-side permuted to match the hT32 partition order
    (hidx), weight cols permuted to the (q, i/f/o/g, c) strip order (gcol).
  - layer biases: layer0 via ones-row appended to x^T (K=65 first chunk);
    layer1 via a K=1 ones-row matmul.
  - layer1 consumes h0T chunks directly (no separate bulk input projection);
    its bias/h0 chunks accumulate early, only the 4 h1 chunks sit on the
    serial recurrence path. Layer0 runs LAG steps ahead so the two chains
    pipeline across all engines.
  - elementwise: ACT sigmoid on [128,384] (i,f,o), tanh on [128,128] (g),
    cell update on DVE in f32 [128,128], h in bf16.
"""

import numpy as np
import ml_dtypes

import concourse.bass as bass
import concourse.mybir as mybir
import concourse.tile as tile
from concourse.bass_utils import run_bass_kernel_spmd

BF16 = mybir.dt.bfloat16
F32 = mybir.dt.float32

B, T, D, H, O = 256, 512, 64, 512, 1
G = 4 * H
NCORES = 8
BL = B // NCORES  # 32
SIG = mybir.ActivationFunctionType.Sigmoid
TANH = mybir.ActivationFunctionType.Tanh

LAG = 8      # layer0 runs this many steps ahead of layer1
DEPTH = 16   # h0T ring depth (must exceed LAG + in-flight margin)


def _split_excess_waits(nc, max_waits: int = 1) -> int:
    """This container's walrus rejects >1 sync wait per instruction; move
    excess waits onto preceding same-engine NOPs (same-engine earlier wait
    is ordering-equivalent)."""
    n_split = 0
    for f in nc.m.functions:
        for bb in f.blocks:
            new_insts = []
            for inst in bb.instructions:
                si = inst.sync_info
                if si is not None and si.on_wait and len(si.on_wait) > max_waits:
                    waits = list(si.on_wait)
                    while len(waits) > max_waits:
                        chunk, waits = waits[:max_waits], waits[max_waits:]
                        nop = mybir.InstNoOp(
                            name=f"{inst.name}-wsplit-{n_split}", ins=[], outs=[]
                        )
                        nop.engine = inst.engine
                        nop.sync_info = mybir.SyncInfo(on_wait=chunk, on_update=[])
                        new_insts.append(nop)
                        n_split += 1
                    si.on_wait = waits
                new_insts.append(inst)
            bb.instructions[:] = new_insts
    return n_split


def build_lstm_nc(t_steps: int = T):
    nc = bass.Bass("TRN2")

    xt_d = nc.dram_tensor("xt", [D + 1, t_steps, BL], BF16, kind="ExternalInput")
    w0x_d = nc.dram_tensor("w0x", [D + 1, G], BF16, kind="ExternalInput")
    w0r_d = nc.dram_tensor("w0r", [128, 4, G], BF16, kind="ExternalInput")
    w1b_d = nc.dram_tensor("w1b", [1, G], BF16, kind="ExternalInput")
    w1x_d = nc.dram_tensor("w1x", [128, 4, G], BF16, kind="ExternalInput")
    w1r_d = nc.dram_tensor("w1r", [128, 4, G], BF16, kind="ExternalInput")
    fcw_d = nc.dram_tensor("fcw", [128, 4], BF16, kind="ExternalInput")
    fcb_d = nc.dram_tensor("fcb", [1, 1], F32, kind="ExternalInput")
    y_d = nc.dram_tensor("y", [BL, O], F32, kind="ExternalOutput")

    with tile.TileContext(nc) as tc:
        with (
            tc.tile_pool(name="singles", bufs=1) as singles,
            tc.tile_pool(name="state", bufs=1) as state,
            tc.tile_pool(name="hring", bufs=DEPTH) as hring,
            tc.tile_pool(name="h1ring", bufs=3) as h1ring,
            tc.tile_pool(name="work", bufs=3) as work,
            tc.tile_pool(name="psumg", bufs=3, space="PSUM") as psumg,
            tc.tile_pool(name="psumfc", bufs=1, space="PSUM") as psumfc,
        ):
            # --- resident constants (DMA order = first-use order) ---
            w0x_s = singles.tile([D + 1, G], BF16)
            nc.sync.dma_start(out=w0x_s, in_=w0x_d[:, :])
            w0r_s = singles.tile([128, 4, G], BF16)
            nc.sync.dma_start(out=w0r_s, in_=w0r_d[:, :, :])
            xt_s = singles.tile([D + 1, t_steps, BL], BF16)
            nc.sync.dma_start(out=xt_s, in_=xt_d[:, :, :])
            w1b_s = singles.tile([1, G], BF16)
            nc.sync.dma_start(out=w1b_s, in_=w1b_d[:, :])
            w1x_s = singles.tile([128, 4, G], BF16)
            nc.sync.dma_start(out=w1x_s, in_=w1x_d[:, :, :])
            w1r_s = singles.tile([128, 4, G], BF16)
            nc.sync.dma_start(out=w1r_s, in_=w1r_d[:, :, :])
            fcw_s = singles.tile([128, 4], BF16)
            nc.sync.dma_start(out=fcw_s, in_=fcw_d[:, :])
            fcb_s = singles.tile([BL, 1], F32)
            nc.sync.dma_start(out=fcb_s, in_=fcb_d[:, :].to_broadcast((BL, 1)))
            ones_r = singles.tile([1, BL], BF16)
            nc.vector.memset(ones_r, 1.0)
            hz = singles.tile([128, 4 * BL], BF16)  # zero initial hT32
            nc.vector.memset(hz, 0.0)

            # --- recurrent cell state ---
            c0 = state.tile([128, 128], F32)
            c1 = state.tile([128, 128], F32)
            nc.vector.memset(c0, 0.0)
            nc.vector.memset(c1, 0.0)

            def emit_mms(gp, first_lhsT, first_rhs, kchunks):
                """Chunk-major column-tiled matmuls into gp [128,512].

                first = (lhsT, rhs_full[*,G]); kchunks = [(hT32_tile, w_s, jj)].
                """
                for q in range(4):
                    nc.tensor.matmul(
                        gp[32 * q : 32 * q + 32, :],
                        first_lhsT,
                        first_rhs[:, 512 * q : 512 * q + 512],
                        start=True, stop=False, tile_position=(0, 32 * q),
                    )
                for ci, (hT, w_s, jj) in enumerate(kchunks):
                    last = ci == len(kchunks) - 1
                    for q in range(4):
                        nc.tensor.matmul(
                            gp[32 * q : 32 * q + 32, :],
                            hT[:, 32 * jj : 32 * jj + 32],
                            w_s[:, jj, 512 * q : 512 * q + 512],
                            start=False, stop=last, tile_position=(0, 32 * q),
                        )

            def elementwise(gp, cell, hT_out, layer):
                sig_ifo = work.tile([128, 384], F32, tag=f"sig{layer}")
                nc.scalar.activation(sig_ifo, gp[:, 0:384], SIG)
                tanh_g = work.tile([128, 128], F32, tag=f"tg{layer}")
                nc.scalar.activation(tanh_g, gp[:, 384:512], TANH)
                ig = work.tile([128, 128], F32, tag=f"ig{layer}")
                nc.vector.tensor_mul(ig, tanh_g, sig_ifo[:, 0:128])
                nc.vector.tensor_mul(cell, cell, sig_ifo[:, 128:256])
                nc.vector.tensor_add(cell, cell, ig)
                tanh_c = work.tile([128, 128], F32, tag=f"tc{layer}")
                nc.scalar.activation(tanh_c, cell, TANH)
                h_new = work.tile([128, 128], BF16, tag=f"hn{layer}")
                nc.vector.tensor_mul(h_new, sig_ifo[:, 256:384], tanh_c)
                nc.vector.transpose(hT_out, h_new)

            h0T_hist = {}
            h1T_prev = hz

            for tt in range(t_steps + LAG):
                if tt < t_steps:
                    # --- layer0 step tt ---
                    prev = h0T_hist.get(tt - 1, hz)
                    gp0 = psumg.tile([128, 512], F32, tag="g0")
                    emit_mms(
                        gp0, xt_s[:, tt, :], w0x_s,
                        [(prev, w0r_s, jj) for jj in range(4)],
                    )
                    h0T = hring.tile([128, 4 * BL], BF16, tag="h0T")
                    elementwise(gp0, c0, h0T, 0)
                    h0T_hist[tt] = h0T
                    h0T_hist.pop(tt - DEPTH, None)
                if tt >= LAG:
                    # --- layer1 step t1 ---
                    t1 = tt - LAG
                    gp1 = psumg.tile([128, 512], F32, tag="g1")
                    emit_mms(
                        gp1, ones_r, w1b_s,
                        [(h0T_hist[t1], w1x_s, jj) for jj in range(4)]
                        + [(h1T_prev, w1r_s, jj) for jj in range(4)],
                    )
                    h1T = h1ring.tile([128, 4 * BL], BF16, tag="h1T")
                    elementwise(gp1, c1, h1T, 1)
                    h1T_prev = h1T

            # --- fc on last h1 ---
            fcp = psumfc.tile([BL, O], F32, tag="fc")
            for jj in range(4):
                nc.tensor.matmul(
                    fcp,
                    h1T_prev[:, 32 * jj : 32 * jj + 32],
                    fcw_s[:, jj : jj + 1],
                    start=(jj == 0), stop=(jj == 3), tile_position=(0, 0),
                )
            y_s = work.tile([BL, O], F32, tag="y")
            nc.vector.tensor_add(y_s, fcp, fcb_s)
            nc.sync.dma_start(out=y_d[:, :], in_=y_s)

    _split_excess_waits(nc)
    return nc


def _perm_indices():
    P = np.arange(128)
    JJ = np.arange(4)
    hidx = (P[:, None] // 32) * 128 + JJ[None, :] * 32 + (P[:, None] % 32)  # [128,4]
    sn = np.arange(512)
    tg = np.array([0, 1, 3, 2])[sn // 128]  # strip order (i,f,o,g) -> torch (i,f,g,o)
    q = np.arange(4)
    gcol = (tg[None, :] * 512 + q[:, None] * 128 + (sn % 128)[None, :]).reshape(-1)
    return hidx, gcol


def prep_inputs(x, w_ih_0, w_hh_0, b_ih_0, b_hh_0, w_ih_1, w_hh_1, b_ih_1, b_hh_1,
                fc_w, fc_b, t_steps: int = T):
    """Host-side layout prep + sharding. Returns per-core in_maps."""
    bf = ml_dtypes.bfloat16
    hidx, gcol = _perm_indices()

    w0x = np.concatenate(
        [w_ih_0[gcol, :].T, (b_ih_0 + b_hh_0)[gcol][None, :]], axis=0
    ).astype(bf)  # [65, G]
    w0r = w_hh_0[gcol[None, None, :], hidx[:, :, None]].astype(bf)  # [128,4,G]
    w1b = (b_ih_1 + b_hh_1)[gcol][None, :].astype(bf)  # [1, G]
    w1x = w_ih_1[gcol[None, None, :], hidx[:, :, None]].astype(bf)  # [128,4,G]
    w1r = w_hh_1[gcol[None, None, :], hidx[:, :, None]].astype(bf)  # [128,4,G]
    fcw = fc_w[0, hidx].astype(bf)  # [128, 4]
    fcb = fc_b.reshape(1, 1).astype(np.float32)

    shared = {"w0x": w0x, "w0r": w0r, "w1b": w1b, "w1x": w1x, "w1r": w1r,
              "fcw": fcw, "fcb": fcb}
    in_maps = []
    for cc in range(NCORES):
        xc = x[cc * BL : (cc + 1) * BL, :t_steps, :]  # [32, T, 64]
        xt = np.transpose(xc, (2, 1, 0))  # [64, T, 32]
        xt = np.concatenate([xt, np.ones((1, t_steps, BL), np.float32)], axis=0)
        in_maps.append({"xt": np.ascontiguousarray(xt).astype(bf), **shared})
    return in_maps


_NC_CACHE = {}


def kernel(x, w_ih_0, w_hh_0, b_ih_0, b_hh_0, w_ih_1, w_hh_1, b_ih_1, b_hh_1,
           fc_w, fc_b):
    x = np.asarray(x, np.float32)
    args = [np.asarray(a, np.float32) for a in (
        w_ih_0, w_hh_0, b_ih_0, b_hh_0, w_ih_1, w_hh_1, b_ih_1, b_hh_1, fc_w, fc_b)]
    if T not in _NC_CACHE:
        _NC_CACHE[T] = build_lstm_nc(T)
    nc = _NC_CACHE[T]
    in_maps = prep_inputs(x, *args, t_steps=T)
    res = run_bass_kernel_spmd(nc, in_maps, core_ids=list(range(NCORES)))
    return np.concatenate([res.results[c]["y"] for c in range(NCORES)], axis=0)


# revision 13
# speedup vs baseline: 1.2679x; 1.0330x over previous
"""Trainium2 Bass kernel for a 2-layer LSTM (B=256, T=512, D=64, H=512) + FC on last step.

Sharding: data-parallel over batch - 32 samples per NeuronCore on 8 cores.

Per-core design (all weights/activations SBUF-resident):
  - gates PSUM layout: partition = 32*q + b (q = H-quarter, b = batch), free =
    (gate, c) with gate order (i,f,o,g), c = H-col within quarter. Computed by
    4-way column-tiled matmuls (tile_position=(0,32q)): four concurrent 32-col
    PE tiles, each streaming its own weight columns.
  - gates are split into two PSUM banks per step: half A = (i,f) cols, half B
    = (o,g) cols. sigmoid(i,f) starts after only half the matmul stream, and
    the bank split keeps PE writes and ACT/DVE reads in different banks.
  - tanh is algebraically eliminated from the gate path: host prep scales the
    g-gate weight columns by 2 and the kernel uses tanh(x) = 2*sigmoid(2x)-1,
    fused into two scalar_tensor_tensor ops:
        p = (sig_g - 0.5) * i          (= i*g/2)
        c = 2*p + (c*f)
    so one sigmoid covers i,f (and g right after; o off the critical path).
  - recurrent state h kept as hT32 [128, (jj,b)]: partition p = 32q+cc holds
    H index 128q + 32jj + cc at free offset 32jj+b. Produced each step by DVE
    32x32-block transposes (nc.vector.transpose) of h_new [128=(q,b),
    128=(jj,cc)] - no PE-mode switches, no PE transposes. Weight rows are
    host-side permuted to match (hidx); cols to the strip order (gcol).
  - biases: layer0 via ones-row appended to x^T (K=65 first chunk); layer1
    via a K=1 ones-row matmul.
  - schedule: engine FIFOs are head-of-line blocking, so every sem-gated PE
    group's dependency is produced a full iteration before the FIFO reaches
    it: x-rows open step t+1's bank during step t; layer1's bias+h0 rows are
    emitted one iteration after their h0 is ready; only the 4 h1 rows gate on
    layer1's recurrence. Layer0 runs `lag` steps ahead of layer1.
"""

import contextlib

import numpy as np
import ml_dtypes

import concourse.bass as bass
import concourse.mybir as mybir
import concourse.tile as tile
from concourse.bass_utils import run_bass_kernel_spmd

BF16 = mybir.dt.bfloat16
F32 = mybir.dt.float32

B, T, D, H, O = 256, 512, 64, 512, 1
G = 4 * H
NCORES = 8
BL = B // NCORES  # 32
SIG = mybir.ActivationFunctionType.Sigmoid
TANH = mybir.ActivationFunctionType.Tanh
ADD = mybir.AluOpType.add
MULT = mybir.AluOpType.mult

DEPTH = 16   # h0T ring depth


@contextlib.contextmanager
def _tiled_scheduler_costs():
    """Teach the Tile scheduler's internal sim that column-tiled matmuls run
    ~4-way concurrent on the PE (tile_position col strips), so it orders
    instructions for the machine we actually run on. Scheduling-only; falls
    back silently if the hook isn't available."""
    try:
        import bass_rust as _br
        real = tile.CoreSim

        class _CoreSimHook:
            def __new__(cls, *a, **k):
                sim = real(*a, **k)
                try:
                    def cb(inst, t0, t1):
                        c = _br.compute_instruction_cost(
                            inst, time=t0, pe_busy_start=t1)
                        if (isinstance(inst, mybir.InstMatmult)
                                and inst.tile_position):
                            return (c[0], c[1] / 4.0)
                        return c
                    sim._sim_state.on_inst_cost = cb
                    _tiled_scheduler_costs.hooked += 1
                except Exception:
                    pass
                return sim

        tile.CoreSim = _CoreSimHook
    except Exception:
        yield
        return
    try:
        yield
    finally:
        tile.CoreSim = real


_tiled_scheduler_costs.hooked = 0


def _split_excess_waits(nc, max_waits: int = 1) -> int:
    """This container's walrus rejects >1 sync wait per instruction; move
    excess waits onto preceding same-engine NOPs (same-engine earlier wait
    is ordering-equivalent)."""
    n_split = 0
    for f in nc.m.functions:
        for bb in f.blocks:
            new_insts = []
            for inst in bb.instructions:
                si = inst.sync_info
                if si is not None and si.on_wait and len(si.on_wait) > max_waits:
                    waits = list(si.on_wait)
                    while len(waits) > max_waits:
                        chunk, waits = waits[:max_waits], waits[max_waits:]
                        nop = mybir.InstNoOp(
                            name=f"{inst.name}-wsplit-{n_split}", ins=[], outs=[]
                        )
                        nop.engine = inst.engine
                        nop.sync_info = mybir.SyncInfo(on_wait=chunk, on_update=[])
                        new_insts.append(nop)
                        n_split += 1
                    si.on_wait = waits
                new_insts.append(inst)
            bb.instructions[:] = new_insts
    return n_split


def build_lstm_nc(t_steps: int = T, lag: int = 2, tail_split: int = 1,
                  depth: int = DEPTH, only_l0: bool = False):
    with _tiled_scheduler_costs():
        return _build_inner(t_steps, lag=lag, tail_split=tail_split,
                            depth=depth, only_l0=only_l0)


def _build_inner(t_steps, *, lag, tail_split, depth, only_l0):
    nc = bass.Bass("TRN2")

    xt_d = nc.dram_tensor("xt", [D + 1, t_steps, BL], BF16, kind="ExternalInput")
    w0x_d = nc.dram_tensor("w0x", [D + 1, G], BF16, kind="ExternalInput")
    w0r_d = nc.dram_tensor("w0r", [128, 4, G], BF16, kind="ExternalInput")
    w1b_d = nc.dram_tensor("w1b", [1, G], BF16, kind="ExternalInput")
    w1x_d = nc.dram_tensor("w1x", [128, 4, G], BF16, kind="ExternalInput")
    w1r_d = nc.dram_tensor("w1r", [128, 4, G], BF16, kind="ExternalInput")
    fcw_d = nc.dram_tensor("fcw", [128, 4], BF16, kind="ExternalInput")
    fcb_d = nc.dram_tensor("fcb", [1, 1], F32, kind="ExternalInput")
    y_d = nc.dram_tensor("y", [BL, O], F32, kind="ExternalOutput")

    with tile.TileContext(nc) as tc:
        with (
            tc.tile_pool(name="singles", bufs=1) as singles,
            tc.tile_pool(name="state", bufs=1) as state,
            tc.tile_pool(name="hring", bufs=depth) as hring,
            tc.tile_pool(name="h1ring", bufs=3) as h1ring,
            tc.tile_pool(name="work", bufs=3) as work,
            tc.tile_pool(name="psumg", bufs=2, space="PSUM") as psumg,
        ):
            # --- resident constants (DMA order = first-use order) ---
            w0x_s = singles.tile([D + 1, G], BF16)
            nc.sync.dma_start(out=w0x_s, in_=w0x_d[:, :])
            w0r_s = singles.tile([128, 4, G], BF16)
            nc.sync.dma_start(out=w0r_s, in_=w0r_d[:, :, :])
            xt_s = singles.tile([D + 1, t_steps, BL], BF16)
            nc.sync.dma_start(out=xt_s, in_=xt_d[:, :, :])
            w1b_s = singles.tile([1, G], BF16)
            nc.sync.dma_start(out=w1b_s, in_=w1b_d[:, :])
            w1x_s = singles.tile([128, 4, G], BF16)
            nc.sync.dma_start(out=w1x_s, in_=w1x_d[:, :, :])
            w1r_s = singles.tile([128, 4, G], BF16)
            nc.sync.dma_start(out=w1r_s, in_=w1r_d[:, :, :])
            fcw_s = singles.tile([128, 4], BF16)
            nc.sync.dma_start(out=fcw_s, in_=fcw_d[:, :])
            fcb_s = singles.tile([BL, 1], F32)
            nc.sync.dma_start(out=fcb_s, in_=fcb_d[:, :].to_broadcast((BL, 1)))
            ones_r = singles.tile([1, BL], BF16)
            nc.vector.memset(ones_r, 1.0)
            hz = singles.tile([128, 4 * BL], BF16)  # zero initial hT32
            nc.vector.memset(hz, 0.0)

            # --- recurrent cell state ---
            c0 = state.tile([128, 128], F32)
            c1 = state.tile([128, 128], F32)
            nc.vector.memset(c0, 0.0)
            nc.vector.memset(c1, 0.0)

            def emit_rows(gps, first, kchunks, start, stop):
                """Column-tiled matmul rows into the two half-banks.

                gps = (gpA, gpB) [128,256] tiles (cols i,f | o,g per strip).
                first = (lhsT, rhs[*, G]) or None; kchunks = [(hT, w_s, jj)].
                Emission: all half-A rows (chunk-major), then half-B.
                """
                for hh, gp in enumerate(gps):
                    off = 256 * hh
                    st = start
                    if first is not None:
                        lhsT, rhs = first
                        for q in range(4):
                            nc.tensor.matmul(
                                gp[32 * q : 32 * q + 32, :],
                                lhsT,
                                rhs[:, 512 * q + off : 512 * q + off + 256],
                                start=st, stop=False,
                                tile_position=(0, 32 * q),
                            )
                        st = False
                    for ci, (hT, w_s, jj) in enumerate(kchunks):
                        last = stop and ci == len(kchunks) - 1
                        for q in range(4):
                            nc.tensor.matmul(
                                gp[32 * q : 32 * q + 32, :],
                                hT[:, 32 * jj : 32 * jj + 32],
                                w_s[:, jj, 512 * q + off : 512 * q + off + 256],
                                start=st and ci == 0, stop=last,
                                tile_position=(0, 32 * q),
                            )

            def elementwise(gpA, gpB, cell, hT_out, layer):
                # gpA cols = (i, f) x 128; gpB cols = (o, g) x 128
                sig_if = work.tile([128, 256], F32, tag=f"sif{layer}")
                nc.scalar.activation(sig_if, gpA, SIG)
                sig_g = work.tile([128, 128], F32, tag=f"sg{layer}")
                nc.scalar.activation(sig_g, gpB[:, 128:256], SIG)
                sig_o = work.tile([128, 128], BF16, tag=f"so{layer}")
                nc.scalar.activation(sig_o, gpB[:, 0:128], SIG)
                cf = work.tile([128, 128], F32, tag=f"cf{layer}")
                tanh_c = work.tile([128, 128], BF16, tag=f"tc{layer}")
                h_new = work.tile([128, 128], BF16, tag=f"hn{layer}")
                p = work.tile([128, 128], F32, tag=f"p{layer}")
                hw_ = 128 // tail_split
                for u in range(tail_split):
                    a = slice(u * hw_, (u + 1) * hw_)
                    nc.vector.tensor_mul(
                        cf[:, a], cell[:, a],
                        sig_if[:, 128 + u * hw_ : 128 + (u + 1) * hw_])
                    # p = (sig_g - 0.5) * i  (= i*g/2)
                    nc.vector.scalar_tensor_tensor(
                        p[:, a], sig_g[:, a], -0.5, sig_if[:, a],
                        op0=ADD, op1=MULT)
                    # c = 2*p + c*f
                    nc.vector.scalar_tensor_tensor(
                        cell[:, a], p[:, a], 2.0, cf[:, a],
                        op0=MULT, op1=ADD)
                    nc.scalar.activation(tanh_c[:, a], cell[:, a], TANH)
                    nc.vector.tensor_mul(h_new[:, a], sig_o[:, a], tanh_c[:, a])
                    nc.vector.transpose(hT_out[:, a], h_new[:, a])

            def open_g0(step):
                gpA = psumg.tile([128, 256], F32, tag="g0A")
                gpB = psumg.tile([128, 256], F32, tag="g0B")
                emit_rows((gpA, gpB), (xt_s[:, step, :], w0x_s), [],
                          start=True, stop=False)
                gp0_pend[step] = (gpA, gpB)

            h0T_hist = {}
            gp0_pend = {}
            gp1_pend = {}
            h1T_prev = hz

            for tt in range(t_steps + lag):
                if tt == 0:
                    open_g0(0)
                if 1 <= tt <= t_steps and not only_l0:
                    # layer1 group A (bias + h0 rows) for step tt-1
                    gpA = psumg.tile([128, 256], F32, tag="g1A")
                    gpB = psumg.tile([128, 256], F32, tag="g1B")
                    emit_rows(
                        (gpA, gpB), (ones_r, w1b_s),
                        [(h0T_hist[tt - 1], w1x_s, jj) for jj in range(4)],
                        start=True, stop=False,
                    )
                    gp1_pend[tt - 1] = (gpA, gpB)
                if tt < t_steps:
                    # --- layer0 step tt: close groups with h rows ---
                    prev = h0T_hist.get(tt - 1, hz)
                    gpA, gpB = gp0_pend.pop(tt)
                    emit_rows(
                        (gpA, gpB), None,
                        [(prev, w0r_s, jj) for jj in range(4)],
                        start=False, stop=True,
                    )
                    if tt + 1 < t_steps:
                        open_g0(tt + 1)
                    h0T = hring.tile([128, 4 * BL], BF16, tag="h0T")
                    elementwise(gpA, gpB, c0, h0T, 0)
                    h0T_hist[tt] = h0T
                    h0T_hist.pop(tt - depth, None)
                if tt >= lag and not only_l0:
                    # --- layer1 step t1: close groups with h1 rows ---
                    t1 = tt - lag
                    gpA, gpB = gp1_pend.pop(t1)
                    emit_rows(
                        (gpA, gpB), None,
                        [(h1T_prev, w1r_s, jj) for jj in range(4)],
                        start=False, stop=True,
                    )
                    h1T = h1ring.tile([128, 4 * BL], BF16, tag="h1T")
                    elementwise(gpA, gpB, c1, h1T, 1)
                    h1T_prev = h1T

            # --- fc on last h1 ---
            fcp = psumg.tile([BL, O], F32, tag="g0A")
            for jj in range(4):
                nc.tensor.matmul(
                    fcp,
                    h1T_prev[:, 32 * jj : 32 * jj + 32],
                    fcw_s[:, jj : jj + 1],
                    start=(jj == 0), stop=(jj == 3), tile_position=(0, 0),
                )
            y_s = work.tile([BL, O], F32, tag="y")
            nc.vector.tensor_add(y_s, fcp, fcb_s)
            nc.sync.dma_start(out=y_d[:, :], in_=y_s)

    _split_excess_waits(nc)
    return nc


def _perm_indices():
    P = np.arange(128)
    JJ = np.arange(4)
    hidx = (P[:, None] // 32) * 128 + JJ[None, :] * 32 + (P[:, None] % 32)  # [128,4]
    sn = np.arange(512)
    tg = np.array([0, 1, 3, 2])[sn // 128]  # strip order (i,f,o,g) -> torch (i,f,g,o)
    q = np.arange(4)
    gcol = (tg[None, :] * 512 + q[:, None] * 128 + (sn % 128)[None, :]).reshape(-1)
    # scale-by-2 for the g gate columns (tanh(x) = 2*sigmoid(2x) - 1)
    gscale = np.where((np.tile(sn, 4) // 128) == 3, 2.0, 1.0).astype(np.float32)
    return hidx, gcol, gscale


def prep_inputs(x, w_ih_0, w_hh_0, b_ih_0, b_hh_0, w_ih_1, w_hh_1, b_ih_1, b_hh_1,
                fc_w, fc_b, t_steps: int = T):
    """Host-side layout prep + sharding. Returns per-core in_maps."""
    bf = ml_dtypes.bfloat16
    hidx, gcol, gs = _perm_indices()

    w0x = (np.concatenate(
        [w_ih_0[gcol, :].T, (b_ih_0 + b_hh_0)[gcol][None, :]], axis=0
    ) * gs[None, :]).astype(bf)  # [65, G]
    w0r = (w_hh_0[gcol[None, None, :], hidx[:, :, None]]
           * gs[None, None, :]).astype(bf)  # [128,4,G]
    w1b = ((b_ih_1 + b_hh_1)[gcol][None, :] * gs[None, :]).astype(bf)  # [1, G]
    w1x = (w_ih_1[gcol[None, None, :], hidx[:, :, None]]
           * gs[None, None, :]).astype(bf)  # [128,4,G]
    w1r = (w_hh_1[gcol[None, None, :], hidx[:, :, None]]
           * gs[None, None, :]).astype(bf)  # [128,4,G]
    fcw = fc_w[0, hidx].astype(bf)  # [128, 4]
    fcb = fc_b.reshape(1, 1).astype(np.float32)

    shared = {"w0x": w0x, "w0r": w0r, "w1b": w1b, "w1x": w1x, "w1r": w1r,
              "fcw": fcw, "fcb": fcb}
    in_maps = []
    for cc in range(NCORES):
        xc = x[cc * BL : (cc + 1) * BL, :t_steps, :]  # [32, T, 64]
        xt = np.transpose(xc, (2, 1, 0))  # [64, T, 32]
        xt = np.concatenate([xt, np.ones((1, t_steps, BL), np.float32)], axis=0)
        in_maps.append({"xt": np.ascontiguousarray(xt).astype(bf), **shared})
    return in_maps


_NC_CACHE = {}


def kernel(x, w_ih_0, w_hh_0, b_ih_0, b_hh_0, w_ih_1, w_hh_1, b_ih_1, b_hh_1,
           fc_w, fc_b):
    x = np.asarray(x, np.float32)
    args = [np.asarray(a, np.float32) for a in (
        w_ih_0, w_hh_0, b_ih_0, b_hh_0, w_ih_1, w_hh_1, b_ih_1, b_hh_1, fc_w, fc_b)]
    if T not in _NC_CACHE:
        _NC_CACHE[T] = build_lstm_nc(T)
    nc = _NC_CACHE[T]
    in_maps = prep_inputs(x, *args, t_steps=T)
    res = run_bass_kernel_spmd(nc, in_maps, core_ids=list(range(NCORES)))
    return np.concatenate([res.results[c]["y"] for c in range(NCORES)], axis=0)


# revision 14
# speedup vs baseline: 1.2889x; 1.0165x over previous
"""Trainium2 Bass kernel for a 2-layer LSTM (B=256, T=512, D=64, H=512) + FC on last step.

Sharding: data-parallel over batch - 32 samples per NeuronCore on 8 cores.

Per-core design (all weights/activations SBUF-resident):
  - gates PSUM layout: partition = 32*q + b (q = H-quarter, b = batch), free =
    (gate, c) with gate order (i,f,o,g), c = H-col within quarter. Computed by
    4-way column-tiled matmuls (tile_position=(0,32q)): four concurrent 32-col
    PE tiles, each streaming its own weight columns.
  - gates are split into two PSUM banks per step: half A = (i,f) cols, half B
    = (o,g) cols. sigmoid(i,f) starts after only half the matmul stream, and
    the bank split keeps PE writes and ACT/DVE reads in different banks.
  - tanh is algebraically eliminated from the gate path: host prep scales the
    g-gate weight columns by 2 and the kernel uses tanh(x) = 2*sigmoid(2x)-1,
    fused into two scalar_tensor_tensor ops:
        p = (sig_g - 0.5) * i          (= i*g/2)
        c = 2*p + (c*f)
    so one sigmoid covers i,f (and g right after; o off the critical path).
  - recurrent state h kept as hT32 [128, (jj,b)]: partition p = 32q+cc holds
    H index 128q + 32jj + cc at free offset 32jj+b. Produced each step by DVE
    32x32-block transposes (nc.vector.transpose) of h_new [128=(q,b),
    128=(jj,cc)] - no PE-mode switches, no PE transposes. Weight rows are
    host-side permuted to match (hidx); cols to the strip order (gcol).
  - biases: layer0 via ones-row appended to x^T (K=65 first chunk); layer1
    via a K=1 ones-row matmul.
  - schedule: engine FIFOs are head-of-line blocking, so every sem-gated PE
    group's dependency is produced a full iteration before the FIFO reaches
    it: x-rows open step t+1's bank during step t; layer1's bias+h0 rows are
    emitted one iteration after their h0 is ready; only the 4 h1 rows gate on
    layer1's recurrence. Layer0 runs `lag` steps ahead of layer1.
"""

import contextlib

import numpy as np
import ml_dtypes

import concourse.bass as bass
import concourse.mybir as mybir
import concourse.tile as tile
from concourse.bass_utils import run_bass_kernel_spmd

BF16 = mybir.dt.bfloat16
F32 = mybir.dt.float32

B, T, D, H, O = 256, 512, 64, 512, 1
G = 4 * H
NCORES = 8
BL = B // NCORES  # 32
SIG = mybir.ActivationFunctionType.Sigmoid
TANH = mybir.ActivationFunctionType.Tanh
ADD = mybir.AluOpType.add
MULT = mybir.AluOpType.mult

DEPTH = 16   # h0T ring depth


@contextlib.contextmanager
def _tiled_scheduler_costs():
    """Teach the Tile scheduler's internal sim that column-tiled matmuls run
    ~4-way concurrent on the PE (tile_position col strips), so it orders
    instructions for the machine we actually run on. Scheduling-only; falls
    back silently if the hook isn't available."""
    try:
        import bass_rust as _br
        real = tile.CoreSim

        class _CoreSimHook:
            def __new__(cls, *a, **k):
                sim = real(*a, **k)
                try:
                    def cb(inst, t0, t1):
                        c = _br.compute_instruction_cost(
                            inst, time=t0, pe_busy_start=t1)
                        if (isinstance(inst, mybir.InstMatmult)
                                and inst.tile_position):
                            return (c[0], c[1] / 4.0)
                        return c
                    sim._sim_state.on_inst_cost = cb
                    _tiled_scheduler_costs.hooked += 1
                except Exception:
                    pass
                return sim

        tile.CoreSim = _CoreSimHook
    except Exception:
        yield
        return
    try:
        yield
    finally:
        tile.CoreSim = real


_tiled_scheduler_costs.hooked = 0


def _split_excess_waits(nc, max_waits: int = 1) -> int:
    """This container's walrus rejects >1 sync wait per instruction; move
    excess waits onto preceding same-engine NOPs (same-engine earlier wait
    is ordering-equivalent)."""
    n_split = 0
    for f in nc.m.functions:
        for bb in f.blocks:
            new_insts = []
            for inst in bb.instructions:
                si = inst.sync_info
                if si is not None and si.on_wait and len(si.on_wait) > max_waits:
                    waits = list(si.on_wait)
                    while len(waits) > max_waits:
                        chunk, waits = waits[:max_waits], waits[max_waits:]
                        nop = mybir.InstNoOp(
                            name=f"{inst.name}-wsplit-{n_split}", ins=[], outs=[]
                        )
                        nop.engine = inst.engine
                        nop.sync_info = mybir.SyncInfo(on_wait=chunk, on_update=[])
                        new_insts.append(nop)
                        n_split += 1
                    si.on_wait = waits
                new_insts.append(inst)
            bb.instructions[:] = new_insts
    return n_split


def build_lstm_nc(t_steps: int = T, lag: int = 2, tail_split: int = 1,
                  depth: int = DEPTH, only_l0: bool = False):
    with _tiled_scheduler_costs():
        return _build_inner(t_steps, lag=lag, tail_split=tail_split,
                            depth=depth, only_l0=only_l0)


def _build_inner(t_steps, *, lag, tail_split, depth, only_l0):
    nc = bass.Bass("TRN2")

    xt_d = nc.dram_tensor("xt", [D + 1, t_steps, BL], BF16, kind="ExternalInput")
    w0x_d = nc.dram_tensor("w0x", [D + 1, G], BF16, kind="ExternalInput")
    w0r_d = nc.dram_tensor("w0r", [128, 4, G], BF16, kind="ExternalInput")
    w1b_d = nc.dram_tensor("w1b", [1, G], BF16, kind="ExternalInput")
    w1x_d = nc.dram_tensor("w1x", [128, 4, G], BF16, kind="ExternalInput")
    w1r_d = nc.dram_tensor("w1r", [128, 4, G], BF16, kind="ExternalInput")
    fcw_d = nc.dram_tensor("fcw", [128, 4], BF16, kind="ExternalInput")
    fcb_d = nc.dram_tensor("fcb", [1, 1], F32, kind="ExternalInput")
    y_d = nc.dram_tensor("y", [BL, O], F32, kind="ExternalOutput")

    with tile.TileContext(nc) as tc:
        with (
            tc.tile_pool(name="singles", bufs=1) as singles,
            tc.tile_pool(name="state", bufs=1) as state,
            tc.tile_pool(name="hring", bufs=depth) as hring,
            tc.tile_pool(name="h1ring", bufs=3) as h1ring,
            tc.tile_pool(name="work", bufs=3) as work,
            tc.tile_pool(name="psumg", bufs=2, space="PSUM") as psumg,
        ):
            # --- resident constants (DMA order = first-use order) ---
            w0x_s = singles.tile([D + 1, G], BF16)
            nc.sync.dma_start(out=w0x_s, in_=w0x_d[:, :])
            w0r_s = singles.tile([128, 4, G], BF16)
            nc.sync.dma_start(out=w0r_s, in_=w0r_d[:, :, :])
            xt_s = singles.tile([D + 1, t_steps, BL], BF16)
            nc.sync.dma_start(out=xt_s, in_=xt_d[:, :, :])
            w1b_s = singles.tile([1, G], BF16)
            nc.sync.dma_start(out=w1b_s, in_=w1b_d[:, :])
            w1x_s = singles.tile([128, 4, G], BF16)
            nc.sync.dma_start(out=w1x_s, in_=w1x_d[:, :, :])
            w1r_s = singles.tile([128, 4, G], BF16)
            nc.sync.dma_start(out=w1r_s, in_=w1r_d[:, :, :])
            fcw_s = singles.tile([128, 4], BF16)
            nc.sync.dma_start(out=fcw_s, in_=fcw_d[:, :])
            fcb_s = singles.tile([BL, 1], F32)
            nc.sync.dma_start(out=fcb_s, in_=fcb_d[:, :].to_broadcast((BL, 1)))
            ones_r = singles.tile([1, BL], BF16)
            nc.vector.memset(ones_r, 1.0)
            hz = singles.tile([128, 4 * BL], BF16)  # zero initial hT32
            nc.vector.memset(hz, 0.0)

            # --- recurrent cell state ---
            c0 = state.tile([128, 128], F32)
            c1 = state.tile([128, 128], F32)
            nc.vector.memset(c0, 0.0)
            nc.vector.memset(c1, 0.0)

            def emit_rows(gps, first, kchunks, start, stop):
                """Column-tiled matmul rows into the two half-banks.

                gps = (gpA, gpB) [128,256] tiles (cols i,f | o,g per strip).
                first = (lhsT, rhs[*, G]) or None; kchunks = [(hT, w_s, jj)].
                Emission: all half-A rows (chunk-major), then half-B.
                """
                for (off, wid), gp in zip(((0, 384), (384, 128)), gps):
                    st = start
                    if first is not None:
                        lhsT, rhs = first
                        for q in range(4):
                            nc.tensor.matmul(
                                gp[32 * q : 32 * q + 32, :],
                                lhsT,
                                rhs[:, 512 * q + off : 512 * q + off + wid],
                                start=st, stop=False,
                                tile_position=(0, 32 * q),
                            )
                        st = False
                    for ci, (hT, w_s, jj) in enumerate(kchunks):
                        last = stop and ci == len(kchunks) - 1
                        for q in range(4):
                            nc.tensor.matmul(
                                gp[32 * q : 32 * q + 32, :],
                                hT[:, 32 * jj : 32 * jj + 32],
                                w_s[:, jj, 512 * q + off : 512 * q + off + wid],
                                start=st and ci == 0, stop=last,
                                tile_position=(0, 32 * q),
                            )

            def elementwise(gpA, gpB, cell, hT_out, layer):
                # gpA cols = (i, f, g) x 128; gpB cols = (o) x 128
                sig_ifg = work.tile([128, 384], F32, tag=f"sifg{layer}")
                nc.scalar.activation(sig_ifg, gpA, SIG)
                sig_o = work.tile([128, 128], BF16, tag=f"so{layer}")
                nc.scalar.activation(sig_o, gpB, SIG)
                cf = work.tile([128, 128], F32, tag=f"cf{layer}")
                tanh_c = work.tile([128, 128], BF16, tag=f"tc{layer}")
                h_new = work.tile([128, 128], BF16, tag=f"hn{layer}")
                p = work.tile([128, 128], F32, tag=f"p{layer}")
                hw_ = 128 // tail_split
                for u in range(tail_split):
                    a = slice(u * hw_, (u + 1) * hw_)
                    nc.vector.tensor_mul(
                        cf[:, a], cell[:, a],
                        sig_ifg[:, 128 + u * hw_ : 128 + (u + 1) * hw_])
                    # p = (sig_g - 0.5) * i  (= i*g/2)
                    nc.vector.scalar_tensor_tensor(
                        p[:, a], sig_ifg[:, 256 + u * hw_ : 256 + (u + 1) * hw_],
                        -0.5, sig_ifg[:, a], op0=ADD, op1=MULT)
                    # c = 2*p + c*f
                    nc.vector.scalar_tensor_tensor(
                        cell[:, a], p[:, a], 2.0, cf[:, a],
                        op0=MULT, op1=ADD)
                    nc.scalar.activation(tanh_c[:, a], cell[:, a], TANH)
                    nc.vector.tensor_mul(h_new[:, a], sig_o[:, a], tanh_c[:, a])
                    nc.vector.transpose(hT_out[:, a], h_new[:, a])

            def open_g0(step):
                gpA = psumg.tile([128, 384], F32, tag="g0A")
                gpB = psumg.tile([128, 128], F32, tag="g0B")
                emit_rows((gpA, gpB), (xt_s[:, step, :], w0x_s), [],
                          start=True, stop=False)
                gp0_pend[step] = (gpA, gpB)

            h0T_hist = {}
            gp0_pend = {}
            gp1_pend = {}
            h1T_prev = hz

            for tt in range(t_steps + lag):
                if tt == 0:
                    open_g0(0)
                if 1 <= tt <= t_steps and not only_l0:
                    # layer1 group A (bias + h0 rows) for step tt-1
                    gpA = psumg.tile([128, 384], F32, tag="g1A")
                    gpB = psumg.tile([128, 128], F32, tag="g1B")
                    emit_rows(
                        (gpA, gpB), (ones_r, w1b_s),
                        [(h0T_hist[tt - 1], w1x_s, jj) for jj in range(4)],
                        start=True, stop=False,
                    )
                    gp1_pend[tt - 1] = (gpA, gpB)
                if tt < t_steps:
                    # --- layer0 step tt: close groups with h rows ---
                    prev = h0T_hist.get(tt - 1, hz)
                    gpA, gpB = gp0_pend.pop(tt)
                    emit_rows(
                        (gpA, gpB), None,
                        [(prev, w0r_s, jj) for jj in range(4)],
                        start=False, stop=True,
                    )
                    if tt + 1 < t_steps:
                        open_g0(tt + 1)
                    h0T = hring.tile([128, 4 * BL], BF16, tag="h0T")
                    elementwise(gpA, gpB, c0, h0T, 0)
                    h0T_hist[tt] = h0T
                    h0T_hist.pop(tt - depth, None)
                if tt >= lag and not only_l0:
                    # --- layer1 step t1: close groups with h1 rows ---
                    t1 = tt - lag
                    gpA, gpB = gp1_pend.pop(t1)
                    emit_rows(
                        (gpA, gpB), None,
                        [(h1T_prev, w1r_s, jj) for jj in range(4)],
                        start=False, stop=True,
                    )
                    h1T = h1ring.tile([128, 4 * BL], BF16, tag="h1T")
                    elementwise(gpA, gpB, c1, h1T, 1)
                    h1T_prev = h1T

            # --- fc on last h1 ---
            fcp = psumg.tile([BL, O], F32, tag="g0A")
            for jj in range(4):
                nc.tensor.matmul(
                    fcp,
                    h1T_prev[:, 32 * jj : 32 * jj + 32],
                    fcw_s[:, jj : jj + 1],
                    start=(jj == 0), stop=(jj == 3), tile_position=(0, 0),
                )
            y_s = work.tile([BL, O], F32, tag="y")
            nc.vector.tensor_add(y_s, fcp, fcb_s)
            nc.sync.dma_start(out=y_d[:, :], in_=y_s)

    _split_excess_waits(nc)
    return nc


def _perm_indices():
    P = np.arange(128)
    JJ = np.arange(4)
    hidx = (P[:, None] // 32) * 128 + JJ[None, :] * 32 + (P[:, None] % 32)  # [128,4]
    sn = np.arange(512)
    tg = np.array([0, 1, 2, 3])[sn // 128]  # strip order (i,f,g,o) = torch order
    q = np.arange(4)
    gcol = (tg[None, :] * 512 + q[:, None] * 128 + (sn % 128)[None, :]).reshape(-1)
    # scale-by-2 for the g gate columns (tanh(x) = 2*sigmoid(2x) - 1)
    gscale = np.where((np.tile(sn, 4) // 128) == 2, 2.0, 1.0).astype(np.float32)
    return hidx, gcol, gscale


def prep_inputs(x, w_ih_0, w_hh_0, b_ih_0, b_hh_0, w_ih_1, w_hh_1, b_ih_1, b_hh_1,
                fc_w, fc_b, t_steps: int = T):
    """Host-side layout prep + sharding. Returns per-core in_maps."""
    bf = ml_dtypes.bfloat16
    hidx, gcol, gs = _perm_indices()

    w0x = (np.concatenate(
        [w_ih_0[gcol, :].T, (b_ih_0 + b_hh_0)[gcol][None, :]], axis=0
    ) * gs[None, :]).astype(bf)  # [65, G]
    w0r = (w_hh_0[gcol[None, None, :], hidx[:, :, None]]
           * gs[None, None, :]).astype(bf)  # [128,4,G]
    w1b = ((b_ih_1 + b_hh_1)[gcol][None, :] * gs[None, :]).astype(bf)  # [1, G]
    w1x = (w_ih_1[gcol[None, None, :], hidx[:, :, None]]
           * gs[None, None, :]).astype(bf)  # [128,4,G]
    w1r = (w_hh_1[gcol[None, None, :], hidx[:, :, None]]
           * gs[None, None, :]).astype(bf)  # [128,4,G]
    fcw = fc_w[0, hidx].astype(bf)  # [128, 4]
    fcb = fc_b.reshape(1, 1).astype(np.float32)

    shared = {"w0x": w0x, "w0r": w0r, "w1b": w1b, "w1x": w1x, "w1r": w1r,
              "fcw": fcw, "fcb": fcb}
    in_maps = []
    for cc in range(NCORES):
        xc = x[cc * BL : (cc + 1) * BL, :t_steps, :]  # [32, T, 64]
        xt = np.transpose(xc, (2, 1, 0))  # [64, T, 32]
        xt = np.concatenate([xt, np.ones((1, t_steps, BL), np.float32)], axis=0)
        in_maps.append({"xt": np.ascontiguousarray(xt).astype(bf), **shared})
    return in_maps


_NC_CACHE = {}


def kernel(x, w_ih_0, w_hh_0, b_ih_0, b_hh_0, w_ih_1, w_hh_1, b_ih_1, b_hh_1,
           fc_w, fc_b):
    x = np.asarray(x, np.float32)
    args = [np.asarray(a, np.float32) for a in (
        w_ih_0, w_hh_0, b_ih_0, b_hh_0, w_ih_1, w_hh_1, b_ih_1, b_hh_1, fc_w, fc_b)]
    if T not in _NC_CACHE:
        _NC_CACHE[T] = build_lstm_nc(T)
    nc = _NC_CACHE[T]
    in_maps = prep_inputs(x, *args, t_steps=T)
    res = run_bass_kernel_spmd(nc, in_maps, core_ids=list(range(NCORES)))
    return np.concatenate([res.results[c]["y"] for c in range(NCORES)], axis=0)


# revision 17
# speedup vs baseline: 1.2966x; 1.0060x over previous
"""Trainium2 Bass kernel for a 2-layer LSTM (B=256, T=512, D=64, H=512) + FC on last step.

Sharding: data-parallel over batch - 32 samples per NeuronCore on 8 cores.

Per-core design (all weights/activations SBUF-resident):
  - gates PSUM layout: partition = 32*q + b (q = H-quarter, b = batch), free =
    (gate, c) with gate order (i,f,o,g), c = H-col within quarter. Computed by
    4-way column-tiled matmuls (tile_position=(0,32q)): four concurrent 32-col
    PE tiles, each streaming its own weight columns.
  - gates are split into two PSUM banks per step: half A = (i,f) cols, half B
    = (o,g) cols. sigmoid(i,f) starts after only half the matmul stream, and
    the bank split keeps PE writes and ACT/DVE reads in different banks.
  - tanh is algebraically eliminated from the gate path: host prep scales the
    g-gate weight columns by 2 and the kernel uses tanh(x) = 2*sigmoid(2x)-1,
    fused into two scalar_tensor_tensor ops:
        p = (sig_g - 0.5) * i          (= i*g/2)
        c = 2*p + (c*f)
    so one sigmoid covers i,f (and g right after; o off the critical path).
  - recurrent state h kept as hT32 [128, (jj,b)]: partition p = 32q+cc holds
    H index 128q + 32jj + cc at free offset 32jj+b. Produced each step by DVE
    32x32-block transposes (nc.vector.transpose) of h_new [128=(q,b),
    128=(jj,cc)] - no PE-mode switches, no PE transposes. Weight rows are
    host-side permuted to match (hidx); cols to the strip order (gcol).
  - biases: layer0 via ones-row appended to x^T (K=65 first chunk); layer1
    via a K=1 ones-row matmul.
  - schedule: engine FIFOs are head-of-line blocking, so every sem-gated PE
    group's dependency is produced a full iteration before the FIFO reaches
    it: x-rows open step t+1's bank during step t; layer1's bias+h0 rows are
    emitted one iteration after their h0 is ready; only the 4 h1 rows gate on
    layer1's recurrence. Layer0 runs `lag` steps ahead of layer1.
"""

import contextlib

import numpy as np
import ml_dtypes

import concourse.bass as bass
import concourse.mybir as mybir
import concourse.tile as tile
from concourse.bass_utils import run_bass_kernel_spmd

BF16 = mybir.dt.bfloat16
F32 = mybir.dt.float32

B, T, D, H, O = 256, 512, 64, 512, 1
G = 4 * H
NCORES = 8
BL = B // NCORES  # 32
SIG = mybir.ActivationFunctionType.Sigmoid
TANH = mybir.ActivationFunctionType.Tanh
ADD = mybir.AluOpType.add
MULT = mybir.AluOpType.mult

DEPTH = 16   # h0T ring depth


@contextlib.contextmanager
def _tiled_scheduler_costs():
    """Teach the Tile scheduler's internal sim that column-tiled matmuls run
    ~4-way concurrent on the PE (tile_position col strips), so it orders
    instructions for the machine we actually run on. Scheduling-only; falls
    back silently if the hook isn't available."""
    try:
        import bass_rust as _br
        real = tile.CoreSim

        class _CoreSimHook:
            def __new__(cls, *a, **k):
                sim = real(*a, **k)
                try:
                    def cb(inst, t0, t1):
                        c = _br.compute_instruction_cost(
                            inst, time=t0, pe_busy_start=t1)
                        if (isinstance(inst, mybir.InstMatmult)
                                and inst.tile_position):
                            return (c[0], c[1] / 4.0)
                        return c
                    sim._sim_state.on_inst_cost = cb
                    _tiled_scheduler_costs.hooked += 1
                except Exception:
                    pass
                return sim

        tile.CoreSim = _CoreSimHook
    except Exception:
        yield
        return
    try:
        yield
    finally:
        tile.CoreSim = real


_tiled_scheduler_costs.hooked = 0


def _split_excess_waits(nc, max_waits: int = 1) -> int:
    """This container's walrus rejects >1 sync wait per instruction; move
    excess waits onto preceding same-engine NOPs (same-engine earlier wait
    is ordering-equivalent)."""
    n_split = 0
    for f in nc.m.functions:
        for bb in f.blocks:
            new_insts = []
            for inst in bb.instructions:
                si = inst.sync_info
                if si is not None and si.on_wait and len(si.on_wait) > max_waits:
                    waits = list(si.on_wait)
                    while len(waits) > max_waits:
                        chunk, waits = waits[:max_waits], waits[max_waits:]
                        nop = mybir.InstNoOp(
                            name=f"{inst.name}-wsplit-{n_split}", ins=[], outs=[]
                        )
                        nop.engine = inst.engine
                        nop.sync_info = mybir.SyncInfo(on_wait=chunk, on_update=[])
                        new_insts.append(nop)
                        n_split += 1
                    si.on_wait = waits
                new_insts.append(inst)
            bb.instructions[:] = new_insts
    return n_split


def build_lstm_nc(t_steps: int = T, lag: int = 2, tail_split: int = 1,
                  depth: int = DEPTH, only_l0: bool = False):
    with _tiled_scheduler_costs():
        return _build_inner(t_steps, lag=lag, tail_split=tail_split,
                            depth=depth, only_l0=only_l0)


def _build_inner(t_steps, *, lag, tail_split, depth, only_l0):
    nc = bass.Bass("TRN2")

    xt_d = nc.dram_tensor("xt", [D + 1, t_steps, BL], BF16, kind="ExternalInput")
    w0x_d = nc.dram_tensor("w0x", [D + 1, G], BF16, kind="ExternalInput")
    w0r_d = nc.dram_tensor("w0r", [128, 4, G], BF16, kind="ExternalInput")
    w1b_d = nc.dram_tensor("w1b", [1, G], BF16, kind="ExternalInput")
    w1x_d = nc.dram_tensor("w1x", [128, 4, G], BF16, kind="ExternalInput")
    w1r_d = nc.dram_tensor("w1r", [128, 4, G], BF16, kind="ExternalInput")
    fcw_d = nc.dram_tensor("fcw", [128, 4], BF16, kind="ExternalInput")
    fcb_d = nc.dram_tensor("fcb", [1, 1], F32, kind="ExternalInput")
    y_d = nc.dram_tensor("y", [BL, O], F32, kind="ExternalOutput")

    with tile.TileContext(nc) as tc:
        with (
            tc.tile_pool(name="singles", bufs=1) as singles,
            tc.tile_pool(name="state", bufs=1) as state,
            tc.tile_pool(name="hring", bufs=depth) as hring,
            tc.tile_pool(name="h1ring", bufs=3) as h1ring,
            tc.tile_pool(name="work", bufs=3) as work,
            tc.tile_pool(name="psumg", bufs=2, space="PSUM") as psumg,
        ):
            # --- resident constants (DMA order = first-use order) ---
            w0x_s = singles.tile([D + 1, G], BF16)
            nc.sync.dma_start(out=w0x_s, in_=w0x_d[:, :])
            w0r_s = singles.tile([128, 4, G], BF16)
            nc.sync.dma_start(out=w0r_s, in_=w0r_d[:, :, :])
            xt_s = singles.tile([D + 1, t_steps, BL], BF16)
            xt_head = min(32, t_steps)
            nc.sync.dma_start(out=xt_s[:, 0:xt_head, :], in_=xt_d[:, 0:xt_head, :])
            w1b_s = singles.tile([1, G], BF16)
            nc.sync.dma_start(out=w1b_s, in_=w1b_d[:, :])
            w1x_s = singles.tile([128, 4, G], BF16)
            nc.sync.dma_start(out=w1x_s, in_=w1x_d[:, :, :])
            w1r_s = singles.tile([128, 4, G], BF16)
            nc.sync.dma_start(out=w1r_s, in_=w1r_d[:, :, :])
            fcw_s = singles.tile([128, 4], BF16)
            nc.sync.dma_start(out=fcw_s, in_=fcw_d[:, :])
            fcb_s = singles.tile([BL, 1], F32)
            nc.sync.dma_start(out=fcb_s, in_=fcb_d[:, :].to_broadcast((BL, 1)))
            if t_steps > xt_head:
                nc.sync.dma_start(
                    out=xt_s[:, xt_head:, :], in_=xt_d[:, xt_head:, :])
            ones_r = singles.tile([1, BL], BF16)
            nc.vector.memset(ones_r, 1.0)
            hz = singles.tile([128, 4 * BL], BF16)  # zero initial hT32
            nc.vector.memset(hz, 0.0)

            # --- recurrent cell state ---
            c0 = state.tile([128, 128], F32)
            c1 = state.tile([128, 128], F32)
            nc.vector.memset(c0, 0.0)
            nc.vector.memset(c1, 0.0)

            def emit_rows(gps, first, kchunks, start, stop):
                """Column-tiled matmul rows into the two half-banks.

                gps = (gpA, gpB) [128,256] tiles (cols i,f | o,g per strip).
                first = (lhsT, rhs[*, G]) or None; kchunks = [(hT, w_s, jj)].
                Emission: all half-A rows (chunk-major), then half-B.
                """
                for (off, wid), gp in zip(((0, 384), (384, 128)), gps):
                    st = start
                    if first is not None:
                        lhsT, rhs = first
                        for q in range(4):
                            nc.tensor.matmul(
                                gp[32 * q : 32 * q + 32, :],
                                lhsT,
                                rhs[:, 512 * q + off : 512 * q + off + wid],
                                start=st, stop=False,
                                tile_position=(0, 32 * q),
                            )
                        st = False
                    for ci, (hT, w_s, jj) in enumerate(kchunks):
                        last = stop and ci == len(kchunks) - 1
                        for q in range(4):
                            nc.tensor.matmul(
                                gp[32 * q : 32 * q + 32, :],
                                hT[:, 32 * jj : 32 * jj + 32],
                                w_s[:, jj, 512 * q + off : 512 * q + off + wid],
                                start=st and ci == 0, stop=last,
                                tile_position=(0, 32 * q),
                            )

            def elementwise(gpA, gpB, cell, hT_out, layer):
                # gpA cols = (i, f, g) x 128; gpB cols = (o) x 128
                sig_ifg = work.tile([128, 384], F32, tag=f"sifg{layer}")
                nc.scalar.activation(sig_ifg, gpA, SIG)
                sig_o = work.tile([128, 128], BF16, tag=f"so{layer}")
                nc.scalar.activation(sig_o, gpB, SIG)
                cf = work.tile([128, 128], F32, tag=f"cf{layer}")
                tanh_c = work.tile([128, 128], BF16, tag=f"tc{layer}")
                h_new = work.tile([128, 128], BF16, tag=f"hn{layer}")
                p = work.tile([128, 128], F32, tag=f"p{layer}")
                hw_ = 128 // tail_split
                for u in range(tail_split):
                    a = slice(u * hw_, (u + 1) * hw_)
                    nc.vector.tensor_mul(
                        cf[:, a], cell[:, a],
                        sig_ifg[:, 128 + u * hw_ : 128 + (u + 1) * hw_])
                    # p = (sig_g - 0.5) * i  (= i*g/2)
                    nc.vector.scalar_tensor_tensor(
                        p[:, a], sig_ifg[:, 256 + u * hw_ : 256 + (u + 1) * hw_],
                        -0.5, sig_ifg[:, a], op0=ADD, op1=MULT)
                    # c = 2*p + c*f
                    nc.vector.scalar_tensor_tensor(
                        cell[:, a], p[:, a], 2.0, cf[:, a],
                        op0=MULT, op1=ADD)
                    nc.scalar.activation(tanh_c[:, a], cell[:, a], TANH)
                    nc.vector.tensor_mul(h_new[:, a], sig_o[:, a], tanh_c[:, a])
                    nc.vector.transpose(hT_out[:, a], h_new[:, a])

            def open_g0(step):
                gpA = psumg.tile([128, 384], F32, tag="g0A")
                gpB = psumg.tile([128, 128], F32, tag="g0B")
                emit_rows((gpA, gpB), (xt_s[:, step, :], w0x_s), [],
                          start=True, stop=False)
                gp0_pend[step] = (gpA, gpB)

            h0T_hist = {}
            gp0_pend = {}
            gp1_pend = {}
            h1T_prev = hz

            for tt in range(t_steps + lag):
                if tt == 0:
                    open_g0(0)
                if 1 <= tt <= t_steps and not only_l0:
                    # layer1 group A (bias + h0 rows) for step tt-1
                    gpA = psumg.tile([128, 384], F32, tag="g1A")
                    gpB = psumg.tile([128, 128], F32, tag="g1B")
                    emit_rows(
                        (gpA, gpB), (ones_r, w1b_s),
                        [(h0T_hist[tt - 1], w1x_s, jj) for jj in range(4)],
                        start=True, stop=False,
                    )
                    gp1_pend[tt - 1] = (gpA, gpB)
                if tt < t_steps:
                    # --- layer0 step tt: close groups with h rows ---
                    prev = h0T_hist.get(tt - 1, hz)
                    gpA, gpB = gp0_pend.pop(tt)
                    emit_rows(
                        (gpA, gpB), None,
                        [(prev, w0r_s, jj) for jj in range(4)],
                        start=False, stop=True,
                    )
                    if tt + 1 < t_steps:
                        open_g0(tt + 1)
                    h0T = hring.tile([128, 4 * BL], BF16, tag="h0T")
                    elementwise(gpA, gpB, c0, h0T, 0)
                    h0T_hist[tt] = h0T
                    h0T_hist.pop(tt - depth, None)
                if tt >= lag and not only_l0:
                    # --- layer1 step t1: close groups with h1 rows ---
                    t1 = tt - lag
                    gpA, gpB = gp1_pend.pop(t1)
                    emit_rows(
                        (gpA, gpB), None,
                        [(h1T_prev, w1r_s, jj) for jj in range(4)],
                        start=False, stop=True,
                    )
                    h1T = h1ring.tile([128, 4 * BL], BF16, tag="h1T")
                    elementwise(gpA, gpB, c1, h1T, 1)
                    h1T_prev = h1T

            # --- fc on last h1 ---
            fcp = psumg.tile([BL, O], F32, tag="g0A")
            for jj in range(4):
                nc.tensor.matmul(
                    fcp,
                    h1T_prev[:, 32 * jj : 32 * jj + 32],
                    fcw_s[:, jj : jj + 1],
                    start=(jj == 0), stop=(jj == 3), tile_position=(0, 0),
                )
            y_s = work.tile([BL, O], F32, tag="y")
            nc.vector.tensor_add(y_s, fcp, fcb_s)
            nc.sync.dma_start(out=y_d[:, :], in_=y_s)

    _split_excess_waits(nc)
    return nc


def _perm_indices():
    P = np.arange(128)
    JJ = np.arange(4)
    hidx = (P[:, None] // 32) * 128 + JJ[None, :] * 32 + (P[:, None] % 32)  # [128,4]
    sn = np.arange(512)
    tg = np.array([0, 1, 2, 3])[sn // 128]  # strip order (i,f,g,o) = torch order
    q = np.arange(4)
    gcol = (tg[None, :] * 512 + q[:, None] * 128 + (sn % 128)[None, :]).reshape(-1)
    # scale-by-2 for the g gate columns (tanh(x) = 2*sigmoid(2x) - 1)
    gscale = np.where((np.tile(sn, 4) // 128) == 2, 2.0, 1.0).astype(np.float32)
    return hidx, gcol, gscale


def prep_inputs(x, w_ih_0, w_hh_0, b_ih_0, b_hh_0, w_ih_1, w_hh_1, b_ih_1, b_hh_1,
                fc_w, fc_b, t_steps: int = T):
    """Host-side layout prep + sharding. Returns per-core in_maps."""
    bf = ml_dtypes.bfloat16
    hidx, gcol, gs = _perm_indices()

    w0x = (np.concatenate(
        [w_ih_0[gcol, :].T, (b_ih_0 + b_hh_0)[gcol][None, :]], axis=0
    ) * gs[None, :]).astype(bf)  # [65, G]
    w0r = (w_hh_0[gcol[None, None, :], hidx[:, :, None]]
           * gs[None, None, :]).astype(bf)  # [128,4,G]
    w1b = ((b_ih_1 + b_hh_1)[gcol][None, :] * gs[None, :]).astype(bf)  # [1, G]
    w1x = (w_ih_1[gcol[None, None, :], hidx[:, :, None]]
           * gs[None, None, :]).astype(bf)  # [128,4,G]
    w1r = (w_hh_1[gcol[None, None, :], hidx[:, :, None]]
           * gs[None, None, :]).astype(bf)  # [128,4,G]
    fcw = fc_w[0, hidx].astype(bf)  # [128, 4]
    fcb = fc_b.reshape(1, 1).astype(np.float32)

    shared = {"w0x": w0x, "w0r": w0r, "w1b": w1b, "w1x": w1x, "w1r": w1r,
              "fcw": fcw, "fcb": fcb}
    in_maps = []
    for cc in range(NCORES):
        xc = x[cc * BL : (cc + 1) * BL, :t_steps, :]  # [32, T, 64]
        xt = np.transpose(xc, (2, 1, 0))  # [64, T, 32]
        xt = np.concatenate([xt, np.ones((1, t_steps, BL), np.float32)], axis=0)
        in_maps.append({"xt": np.ascontiguousarray(xt).astype(bf), **shared})
    return in_maps


_NC_CACHE = {}


def kernel(x, w_ih_0, w_hh_0, b_ih_0, b_hh_0, w_ih_1, w_hh_1, b_ih_1, b_hh_1,
           fc_w, fc_b):
    x = np.asarray(x, np.float32)
    args = [np.asarray(a, np.float32) for a in (
        w_ih_0, w_hh_0, b_ih_0, b_hh_0, w_ih_1, w_hh_1, b_ih_1, b_hh_1, fc_w, fc_b)]
    if T not in _NC_CACHE:
        _NC_CACHE[T] = build_lstm_nc(T)
    nc = _NC_CACHE[T]
    in_maps = prep_inputs(x, *args, t_steps=T)
    res = run_bass_kernel_spmd(nc, in_maps, core_ids=list(range(NCORES)))
    return np.concatenate([res.results[c]["y"] for c in range(NCORES)], axis=0)
